# revision 1
# baseline (speedup 1.0000x reference)
"""Trainium2 Bass kernel for nn_AxialShift: 5x conv1x1(192->192) + 2x GroupNorm(1,C)
+ exact gelu + 3 axial channel-chunk shifts, data-parallel over batch (1 sample/core,
8 cores). Self-contained: hardcodes shapes (B=8, C=192, R=32)."""

import os
import numpy as np
import ml_dtypes
from contextlib import ExitStack

import concourse.bass as bass
import concourse.tile as tile
from concourse import bacc
from concourse import mybir
from concourse.bass_utils import run_bass_kernel_spmd

C = 192
CA = 128          # channel split A: 0..128 on partitions 0..127
CB = 64           # channel split B: 128..192 on partitions 0..63
R = 32
N = R * R * R     # 32768 flat spatial, n = d*1024 + h*32 + w
T = 512           # free-dim tile (half a D-plane)
NT = int(os.environ.get("KNT", str(N // T)))  # tiles to emit (64 full)
EPS = 1e-5

f32 = mybir.dt.float32
f32r = mybir.dt.float32r
bf16 = mybir.dt.bfloat16
AF = mybir.ActivationFunctionType
ALU = mybir.AluOpType
AX = mybir.AxisListType
GELU = (AF.Tanh if os.environ.get("SIM_TANH") else AF.Gelu)
ST1 = bool(os.environ.get("ST1"))
NOSTATS = bool(os.environ.get("NOSTATS"))


def _conv_mms(nc, psA, psB, wA, wB, rA, rB):
    """psA[128,T] = w[:, :128].T @ r ; psB[64,T] = w[:, 128:].T @ r  (K=192 in 2 steps)."""
    nc.tensor.matmul(psA, wA[:, 0:CA], rA, start=True, stop=False)
    nc.tensor.matmul(psA, wB[:, 0:CA], rB, start=False, stop=True)
    nc.tensor.matmul(psB, wA[:, CA:C], rA, start=True, stop=False)
    nc.tensor.matmul(psB, wB[:, CA:C], rB, start=False, stop=True)


def _build():
    nc = bacc.Bacc("TRN2", target_bir_lowering=False, debug=False, num_devices=8)

    dp = lambda name, shape, dt, kind: nc.dram_tensor(name, shape, dt, kind=kind).ap()
    x_d = dp("x", [C, N], bf16, "ExternalInput")
    w1T_d = dp("w1T", [C, C], bf16, "ExternalInput")
    w22T_d = dp("w22T", [C, C], bf16, "ExternalInput")
    w21T_d = dp("w21T", [C, C], bf16, "ExternalInput")
    w23T_d = dp("w23T", [C, C], bf16, "ExternalInput")
    w3T_d = dp("w3T", [C, C], bf16, "ExternalInput")
    vecs_d = {}
    for nm in ("b1", "b22", "b21", "b23", "b3", "n1w", "n1b", "n2w", "n2b"):
        vecs_d[nm] = dp(nm, [C, 1], f32, "ExternalInput")
    out_d = dp("out", [C, N], f32, "ExternalOutput")
    h1_d = dp("h1buf", [C, N], bf16, "Internal")
    c1_d = dp("c1buf", [C, N], bf16, "Internal")
    c2_d = dp("c2buf", [C, N], bf16, "Internal")
    t_d = dp("tbuf", [C, N], bf16, "Internal")

    with tile.TileContext(nc) as tc, ExitStack() as ctx:
        wp = ctx.enter_context(tc.tile_pool(name="weights", bufs=1))
        vp = ctx.enter_context(tc.tile_pool(name="vecs", bufs=1))
        sp = ctx.enter_context(tc.tile_pool(name="stats", bufs=1))
        io = ctx.enter_context(tc.tile_pool(name="io", bufs=4))
        ev = ctx.enter_context(tc.tile_pool(name="evac", bufs=4))
        scr = ctx.enter_context(tc.tile_pool(name="scratch", bufs=2))
        pm = ctx.enter_context(tc.tile_pool(name="psA", bufs=2, space="PSUM"))
        pb = ctx.enter_context(tc.tile_pool(name="psB", bufs=2, space="PSUM"))
        pt = ctx.enter_context(tc.tile_pool(name="psT", bufs=1, space="PSUM"))

        # ---- weights + per-channel vectors to SBUF ----
        def load_w(d, dt):
            a = wp.tile([CA, C], dt, tag=f"w{d.name}A")
            b = wp.tile([CB, C], dt, tag=f"w{d.name}B")
            eng = nc.gpsimd if dt != d.dtype else nc.sync
            eng.dma_start(a[:], d[0:CA, :])
            eng.dma_start(b[:], d[CA:C, :])
            return a, b

        w1A, w1B = load_w(w1T_d, bf16)
        w22A, w22B = load_w(w22T_d, bf16)
        w21A, w21B = load_w(w21T_d, bf16)
        w23A, w23B = load_w(w23T_d, bf16)
        w3A, w3B = load_w(w3T_d, bf16)

        vecs = {}
        for nm, d in vecs_d.items():
            a = vp.tile([CA, 1], f32, tag=f"v{nm}A")
            b = vp.tile([CB, 1], f32, tag=f"v{nm}B")
            nc.sync.dma_start(a[:], d[0:CA, :])
            nc.sync.dma_start(b[:], d[CA:C, :])
            vecs[nm] = (a, b)

        ones_a = vp.tile([1, CA], f32, tag="onesA")
        ones_b = vp.tile([1, CB], f32, tag="onesB")
        nc.gpsimd.memset(ones_a[:], 1.0)
        nc.gpsimd.memset(ones_b[:], 1.0)

        # PE warmups: absorb weight-DMA semaphore waits before the hot loops
        for wa, wb in ((w1A, w1B), (w22A, w22B), (w21A, w21B),
                       (w23A, w23B), (w3A, w3B)):
            pwA = pt.tile([CA, 1], f32, tag="ptA")
            pwB = pt.tile([CB, 1], f32, tag="ptB")
            nc.tensor.matmul(pwA[:], wa[:, 0:CA], wa[:, 0:1], start=True, stop=True)
            nc.tensor.matmul(pwB[:], wb[:, CA:C], wb[:, 0:1], start=True, stop=True)

        # stats column accumulators: [sum | ssq] per pass
        s1A = sp.tile([CA, NT], f32, tag="s1A")
        q1A = sp.tile([CA, NT], f32, tag="q1A")
        s1B = sp.tile([CB, NT], f32, tag="s1B")
        q1B = sp.tile([CB, NT], f32, tag="q1B")
        s2A = sp.tile([CA, NT], f32, tag="s2A")
        q2A = sp.tile([CA, NT], f32, tag="q2A")
        s2B = sp.tile([CB, NT], f32, tag="s2B")
        q2B = sp.tile([CB, NT], f32, tag="q2B")

        # ---------- Stage 1: h1 = w1 @ x + b1 (fp32r), stats of h1 ----------
        for i in range(NT):
            o = i * T
            xa = io.tile([CA, T], bf16, tag="xa")
            xb = io.tile([CB, T], bf16, tag="xb")
            nc.sync.dma_start(xa[:], x_d[0:CA, o:o + T])
            nc.sync.dma_start(xb[:], x_d[CA:C, o:o + T])
            psA = pm.tile([CA, T], f32)
            psB = pb.tile([CB, T], f32)
            _conv_mms(nc, psA[:], psB[:], w1A[:], w1B[:], xa[:], xb[:])
            hA = ev.tile([CA, T], bf16, tag="hA")
            hB = ev.tile([CB, T], bf16, tag="hB")
            nc.scalar.activation(hA[:], psA[:], AF.Identity, bias=vecs["b1"][0][:])
            nc.scalar.activation(hB[:], psB[:], AF.Identity, bias=vecs["b1"][1][:])
            if not NOSTATS:
                sqA = scr.tile([CA, T], f32, tag="sqA")
                sqB = scr.tile([CB, T], f32, tag="sqB")
                nc.scalar.activation(sqA[:], hA[:], AF.Square)
                nc.scalar.activation(sqB[:], hB[:], AF.Square)
                nc.vector.tensor_reduce(s1A[:, i:i + 1], hA[:], AX.X, ALU.add)
                nc.vector.tensor_reduce(s1B[:, i:i + 1], hB[:], AX.X, ALU.add)
                nc.vector.tensor_reduce(q1A[:, i:i + 1], sqA[:], AX.X, ALU.add)
                nc.vector.tensor_reduce(q1B[:, i:i + 1], sqB[:], AX.X, ALU.add)
            nc.sync.dma_start(h1_d[0:CA, o:o + T], hA[:])
            nc.sync.dma_start(h1_d[CA:C, o:o + T], hB[:])
            if ST1:
                nc.gpsimd.dma_start(out_d[0:CA, o:o + T], hA[:])
                nc.gpsimd.dma_start(out_d[CA:C, o:o + T], hB[:])
        # ---------- stats finalize -> scale/bias vectors for norm1 ----------
        def finalize(sA, qA, sB, qB, nw, nb, tag):
            # per-channel totals
            csA = sp.tile([CA, 1], f32, tag=f"csA{tag}")
            cqA = sp.tile([CA, 1], f32, tag=f"cqA{tag}")
            csB = sp.tile([CB, 1], f32, tag=f"csB{tag}")
            cqB = sp.tile([CB, 1], f32, tag=f"cqB{tag}")
            nc.vector.tensor_reduce(csA[:], sA[:], AX.X, ALU.add)
            nc.vector.tensor_reduce(cqA[:], qA[:], AX.X, ALU.add)
            nc.vector.tensor_reduce(csB[:], sB[:], AX.X, ALU.add)
            nc.vector.tensor_reduce(cqB[:], qB[:], AX.X, ALU.add)
            # cross-partition via DRAM bounce -> (1, 2C) row [sums | ssqs]
            row_d = nc.dram_tensor(f"statrow{tag}", [2 * C], f32, kind="Internal").ap()
            nc.sync.dma_start(row_d[0:CA], csA[:].rearrange("p one -> (p one)"))
            nc.sync.dma_start(row_d[CA:C], csB[:].rearrange("p one -> (p one)"))
            nc.sync.dma_start(row_d[C:C + CA], cqA[:].rearrange("p one -> (p one)"))
            nc.sync.dma_start(row_d[C + CA:2 * C], cqB[:].rearrange("p one -> (p one)"))
            row = sp.tile([1, 2 * C], f32, tag=f"row{tag}")
            nc.sync.dma_start(row[:], row_d[:].rearrange("(one n) -> one n", one=1))
            stot = sp.tile([1, 1], f32, tag=f"stot{tag}")
            qtot = sp.tile([1, 1], f32, tag=f"qtot{tag}")
            nc.vector.tensor_reduce(stot[:], row[:, 0:C], AX.X, ALU.add)
            nc.vector.tensor_reduce(qtot[:], row[:, C:2 * C], AX.X, ALU.add)
            inv = 1.0 / float(C * N)
            mu = sp.tile([1, 1], f32, tag=f"mu{tag}")
            ex2 = sp.tile([1, 1], f32, tag=f"ex2{tag}")
            nc.vector.tensor_scalar_mul(mu[:], stot[:], inv)
            nc.vector.tensor_scalar_mul(ex2[:], qtot[:], inv)
            var = sp.tile([1, 1], f32, tag=f"var{tag}")
            nc.vector.tensor_tensor(var[:], mu[:], mu[:], ALU.mult)
            nc.vector.tensor_tensor(var[:], ex2[:], var[:], ALU.subtract)
            nc.vector.tensor_scalar_add(var[:], var[:], EPS)
            rsq = sp.tile([1, 1], f32, tag=f"rsq{tag}")
            nc.vector.reciprocal(rsq[:], var[:])
            rs = sp.tile([1, 1], f32, tag=f"rs{tag}")
            nc.scalar.activation(rs[:], rsq[:], AF.Sqrt)
            nmu = sp.tile([1, 1], f32, tag=f"nmu{tag}")
            nc.vector.tensor_scalar_mul(nmu[:], mu[:], -1.0)
            # broadcast rs and -mu to (CA,1)/(CB,1) via K=1 matmul with ones
            bc = {}
            for val, vn in ((rs, "rs"), (nmu, "nmu")):
                pA = pt.tile([CA, 1], f32, tag="ptA")
                pB = pt.tile([CB, 1], f32, tag="ptB")
                nc.tensor.matmul(pA[:], ones_a[:], val[:], start=True, stop=True)
                nc.tensor.matmul(pB[:], ones_b[:], val[:], start=True, stop=True)
                tA = sp.tile([CA, 1], f32, tag=f"bc{vn}A{tag}")
                tB = sp.tile([CB, 1], f32, tag=f"bc{vn}B{tag}")
                nc.vector.tensor_copy(tA[:], pA[:])
                nc.vector.tensor_copy(tB[:], pB[:])
                bc[vn] = (tA, tB)
            # scale = rs*nw ; bias = nb + (-mu)*scale
            outs = []
            for half in (0, 1):
                P = CA if half == 0 else CB
                sc = sp.tile([P, 1], f32, tag=f"scale{tag}{half}")
                bi = sp.tile([P, 1], f32, tag=f"bias{tag}{half}")
                nc.vector.tensor_tensor(sc[:], bc["rs"][half][:], nw[half][:], ALU.mult)
                nc.vector.tensor_tensor(bi[:], bc["nmu"][half][:], sc[:], ALU.mult)
                nc.vector.tensor_tensor(bi[:], bi[:], nb[half][:], ALU.add)
                outs += [sc, bi]
            return outs  # scA, biA, scB, biB

        NT2 = 0 if ST1 else NT
        if not ST1:
            sc1A, bi1A, sc1B, bi1B = finalize(s1A, q1A, s1B, q1B,
                                              vecs["n1w"], vecs["n1b"], "1")

        # ---------- shifted-load helper ----------
        # chunk 0 (ch 0..64): shift -1 (reads coord+1, edge top); chunk 1 identity;
        # chunk 2 (ch 128..192): shift +1 (reads coord-1, edge bottom).
        def load_shifted(src, gA, gB, i, axis):
            d, rem = divmod(i * T, 1024)
            h0 = rem // 32
            o = i * T
            if axis == 2:  # D shift: whole-tile plane offset, reflect at d=0/31
                om = o + (1024 if d < R - 1 else -1024)
                op = o - (1024 if d > 0 else -1024)
                nc.sync.dma_start(gA[0:CB, :], src[0:CB, om:om + T])
                nc.sync.dma_start(gA[CB:CA, :], src[CB:CA, o:o + T])
                nc.sync.dma_start(gB[:], src[CA:C, op:op + T])
            elif axis == 3:  # H shift: row offset +-32 within plane, reflect h=0/31
                if h0 == 0:  # rows 0..15
                    nc.sync.dma_start(gA[0:CB, :], src[0:CB, o + 32:o + 32 + T])
                    nc.sync.dma_start(gB[:, 0:32], src[CA:C, o + 32:o + 64])
                    nc.sync.dma_start(gB[:, 32:T], src[CA:C, o:o + T - 32])
                else:        # rows 16..31
                    nc.sync.dma_start(gA[0:CB, 0:T - 32], src[0:CB, o + 32:o + T])
                    nc.sync.dma_start(gA[0:CB, T - 32:T],
                                      src[0:CB, o + 14 * 32:o + 15 * 32])
                    nc.sync.dma_start(gB[:], src[CA:C, o - 32:o - 32 + T])
                nc.sync.dma_start(gA[CB:CA, :], src[CB:CA, o:o + T])
            else:  # axis == 4, W shift: offset +-1 within each 32-row, reflect w=0/31
                s3 = src[0:CB, o:o + T].rearrange("c (r w) -> c r w", w=32)
                g3 = gA[0:CB, :].rearrange("c (r w) -> c r w", w=32)
                nc.sync.dma_start(g3[:, :, 0:31], s3[:, :, 1:32])
                # edge w=31 <- src w=30 (already in-tile at col 29): SBUF copy
                nc.vector.tensor_copy(g3[:, :, 31:32], g3[:, :, 29:30])
                sB3 = src[CA:C, o:o + T].rearrange("c (r w) -> c r w", w=32)
                gB3 = gB[:].rearrange("c (r w) -> c r w", w=32)
                nc.sync.dma_start(gB3[:, :, 1:32], sB3[:, :, 0:31])
                # edge w=0 <- src w=1 (in-tile at col 2): SBUF copy
                nc.vector.tensor_copy(gB3[:, :, 0:1], gB3[:, :, 2:3])
                nc.sync.dma_start(gA[CB:CA, :], src[CB:CA, o:o + T])

        # ---------- Stage 3: c1 = w22 @ shiftH(gelu(norm1(h1))) + b22 ----------
        for i in range(NT2):
            o = i * T
            gA = io.tile([CA, T], bf16, tag="gA")
            gB = io.tile([CB, T], bf16, tag="gB")
            load_shifted(h1_d, gA, gB, i, axis=3)
            aA = io.tile([CA, T], bf16, tag="aA")
            aB = io.tile([CB, T], bf16, tag="aB")
            nc.scalar.activation(aA[:], gA[:], GELU, scale=sc1A[:], bias=bi1A[:])
            nc.scalar.activation(aB[:], gB[:], GELU, scale=sc1B[:], bias=bi1B[:])
            psA = pm.tile([CA, T], f32)
            psB = pb.tile([CB, T], f32)
            _conv_mms(nc, psA[:], psB[:], w22A[:], w22B[:], aA[:], aB[:])
            hA = ev.tile([CA, T], bf16, tag="hA")
            hB = ev.tile([CB, T], bf16, tag="hB")
            nc.scalar.activation(hA[:], psA[:], AF.Identity, bias=vecs["b22"][0][:])
            nc.scalar.activation(hB[:], psB[:], AF.Identity, bias=vecs["b22"][1][:])
            nc.sync.dma_start(c1_d[0:CA, o:o + T], hA[:])
            nc.sync.dma_start(c1_d[CA:C, o:o + T], hB[:])

        # ---------- Stage 4: c2 = w21 @ shiftD(c1) + b21 ----------
        for i in range(NT2):
            o = i * T
            gA = io.tile([CA, T], bf16, tag="gA")
            gB = io.tile([CB, T], bf16, tag="gB")
            load_shifted(c1_d, gA, gB, i, axis=2)
            psA = pm.tile([CA, T], f32)
            psB = pb.tile([CB, T], f32)
            _conv_mms(nc, psA[:], psB[:], w21A[:], w21B[:], gA[:], gB[:])
            hA = ev.tile([CA, T], bf16, tag="hA")
            hB = ev.tile([CB, T], bf16, tag="hB")
            nc.scalar.activation(hA[:], psA[:], AF.Identity, bias=vecs["b21"][0][:])
            nc.scalar.activation(hB[:], psB[:], AF.Identity, bias=vecs["b21"][1][:])
            nc.sync.dma_start(c2_d[0:CA, o:o + T], hA[:])
            nc.sync.dma_start(c2_d[CA:C, o:o + T], hB[:])

        # ---------- Stage 5: t = gelu(w23 @ shiftW(c2) + b23), stats of t ----------
        for i in range(NT2):
            o = i * T
            gA = io.tile([CA, T], bf16, tag="gA")
            gB = io.tile([CB, T], bf16, tag="gB")
            load_shifted(c2_d, gA, gB, i, axis=4)
            psA = pm.tile([CA, T], f32)
            psB = pb.tile([CB, T], f32)
            _conv_mms(nc, psA[:], psB[:], w23A[:], w23B[:], gA[:], gB[:])
            tA = ev.tile([CA, T], bf16, tag="hA")
            tB = ev.tile([CB, T], bf16, tag="hB")
            nc.scalar.activation(tA[:], psA[:], GELU, bias=vecs["b23"][0][:])
            nc.scalar.activation(tB[:], psB[:], GELU, bias=vecs["b23"][1][:])
            sqA = scr.tile([CA, T], f32, tag="sqA")
            sqB = scr.tile([CB, T], f32, tag="sqB")
            nc.scalar.activation(sqA[:], tA[:], AF.Square)
            nc.scalar.activation(sqB[:], tB[:], AF.Square)
            nc.vector.tensor_reduce(s2A[:, i:i + 1], tA[:], AX.X, ALU.add)
            nc.vector.tensor_reduce(s2B[:, i:i + 1], tB[:], AX.X, ALU.add)
            nc.vector.tensor_reduce(q2A[:, i:i + 1], sqA[:], AX.X, ALU.add)
            nc.vector.tensor_reduce(q2B[:, i:i + 1], sqB[:], AX.X, ALU.add)
            nc.sync.dma_start(t_d[0:CA, o:o + T], tA[:])
            nc.sync.dma_start(t_d[CA:C, o:o + T], tB[:])

        # ---------- stats2 finalize; fold norm2 into w3 ----------
        sc2A, bi2A, sc2B, bi2B = ((None,) * 4 if ST1 else
            finalize(s2A, q2A, s2B, q2B, vecs["n2w"], vecs["n2b"], "2"))
        if not ST1:
            w3sA = wp.tile([CA, C], bf16, tag="w3sA")
            w3sB = wp.tile([CB, C], bf16, tag="w3sB")
            nc.vector.tensor_scalar_mul(w3sA[:], w3A[:], sc2A[:])
            nc.vector.tensor_scalar_mul(w3sB[:], w3B[:], sc2B[:])
            b2Ab = sp.tile([CA, 1], bf16, tag="b2Ab")
            b2Bb = sp.tile([CB, 1], bf16, tag="b2Bb")
            nc.vector.tensor_copy(b2Ab[:], bi2A[:])
            nc.vector.tensor_copy(b2Bb[:], bi2B[:])
            pyA = pt.tile([CA, 1], f32, tag="ptA")
            pyB = pt.tile([CB, 1], f32, tag="ptB")
            _conv_mms(nc, pyA[:], pyB[:], w3A[:], w3B[:], b2Ab[:], b2Bb[:])
            ybA = sp.tile([CA, 1], f32, tag="ybA")
            ybB = sp.tile([CB, 1], f32, tag="ybB")
            nc.scalar.activation(ybA[:], pyA[:], AF.Identity, bias=vecs["b3"][0][:])
            nc.scalar.activation(ybB[:], pyB[:], AF.Identity, bias=vecs["b3"][1][:])

        # ---------- Stage 7: out = w3s @ t + yb ----------
        for i in range(NT2):
            o = i * T
            tA = io.tile([CA, T], bf16, tag="gA")
            tB = io.tile([CB, T], bf16, tag="gB")
            nc.sync.dma_start(tA[:], t_d[0:CA, o:o + T])
            nc.sync.dma_start(tB[:], t_d[CA:C, o:o + T])
            psA = pm.tile([CA, T], f32)
            psB = pb.tile([CB, T], f32)
            _conv_mms(nc, psA[:], psB[:], w3sA[:], w3sB[:], tA[:], tB[:])
            oA = ev.tile([CA, T], f32, tag="oA")
            oB = ev.tile([CB, T], f32, tag="oB")
            nc.scalar.activation(oA[:], psA[:], AF.Identity, bias=ybA[:])
            nc.scalar.activation(oB[:], psB[:], AF.Identity, bias=ybB[:])
            nc.sync.dma_start(out_d[0:CA, o:o + T], oA[:])
            nc.sync.dma_start(out_d[CA:C, o:o + T], oB[:])

    nc.finalize()
    return nc


def kernel(x, w1, b1, n1w, n1b, w21, b21, w22, b22, w23, b23, n2w, n2b, w3, b3):
    bf = ml_dtypes.bfloat16
    nc = _build()
    col = lambda v: np.ascontiguousarray(np.asarray(v, np.float32).reshape(C, 1))
    common = {
        "w1T": np.ascontiguousarray(np.asarray(w1, np.float32).T.astype(bf)),
        "w22T": np.ascontiguousarray(np.asarray(w22, np.float32).T.astype(bf)),
        "w21T": np.ascontiguousarray(np.asarray(w21, np.float32).T.astype(bf)),
        "w23T": np.ascontiguousarray(np.asarray(w23, np.float32).T.astype(bf)),
        "w3T": np.ascontiguousarray(np.asarray(w3, np.float32).T.astype(bf)),
        "b1": col(b1), "b22": col(b22), "b21": col(b21), "b23": col(b23),
        "b3": col(b3), "n1w": col(n1w), "n1b": col(n1b),
        "n2w": col(n2w), "n2b": col(n2b),
    }
    xs = np.asarray(x, np.float32).astype(bf)
    in_maps = [dict(common, x=np.ascontiguousarray(xs[i].reshape(C, N)))
               for i in range(8)]
    trace = bool(os.environ.get("KPROF"))
    ncores = int(os.environ.get("NCORES", "8"))
    res = run_bass_kernel_spmd(nc, in_maps[:ncores], core_ids=list(range(ncores)),
                               trace=trace)
    if trace:
        print("HW exec time:", res.exec_time_ns, "ns")
        print("profile trace_dir:", getattr(res, "profile_json", None))
    outs = [np.asarray(res.results[i]["out"], np.float32).reshape(C, R, R, R)
            for i in range(len(res.results))]
    while len(outs) < 8:
        outs.append(outs[0])
    return np.stack(outs)

